# revision 14
# baseline (speedup 1.0000x reference)
"""AttnBlock (GroupNorm + spatial self-attention + proj + residual) on 8 TRN2 cores.

Problem shapes (hardcoded): x (4, 512, 64, 64) fp32, 1x1-conv weights (512, 512).

Sharding: 8 cores = (batch b in 0..3) x (query half qh in 0..1). Attention is
permutation-invariant over key positions, so each core receives its batch's
x rotated along the flattened spatial axis so that its own 2048 query
positions are always columns 0:2048 -- the compiled NEFF is identical on all
cores (pure SPMD, no collectives). Each core computes GroupNorm, k~ and u for
all 4096 positions, attention for its 2048 queries and the residual,
producing a (512, 2048) fp32 shard.

Fast path (bq == bk == 0, true for this problem): all heavy matmuls run as
fp8e4 DoubleRow (256-deep contraction per instruction, ~2x PE throughput).
Two host-side weight merges remove two of the four convs:
  k~ = (Wq^T Wk) h   so scores = k~^T h (q conv gone; softmax is invariant
                     to the per-query constant)
  u  = (Wp Wv) h + (Wp bv)  since the proj conv commutes with the attention
                     average: out = x + (sum_j a_ij u_j)/S_i + bp2
Merged weights are pre-scaled by 16 to keep their entries out of the fp8e4
subnormal range; the 1/16 folds into the exp scale and into the softmax
denominator (whose all-ones matmul stationary becomes all-16s). The
denominator itself is accumulated on the PE by an extra DoubleRow matmul per
key pair whose [128, 2, 128] stationary is constant 16.0 -- the matmul both
reduces over partitions and broadcasts S to all 128 output partitions, so
the epilogue is just two DVE ops per tile: fin = att_psum * (1/S) + x, with
x reused from the SBUF-resident staging tiles (no residual re-read from HBM).

Numerics: GroupNorm/softmax statistics in fp32, fp32 PSUM accumulation
everywhere; fp8 only on matmul operands. Measured end-to-end rel err ~4e-3.

A general fallback (separate q/k convs, fp16 operands, nonzero biases) is
kept and selected automatically when bq/bk are nonzero.
"""

from contextlib import ExitStack

import ml_dtypes
import numpy as np

import concourse.bacc as bacc
import concourse.mybir as mybir
import concourse.tile as tile
from concourse.bass_utils import run_bass_kernel_spmd

F32 = mybir.dt.float32
F16 = mybir.dt.float16
F8 = mybir.dt.float8e4

C = 512          # channels
N = 4096         # spatial positions (64*64)
NQ = 2048        # query positions per core
P = 128          # partitions
CT = C // P      # 4 channel tiles
NB = 512         # matmul free-dim block
NJ = N // P      # 32 key tiles
G = 32           # groups
GS = C // G      # 16 channels per group
GPT = P // GS    # 8 groups per channel tile
EPS = 1e-6
SCALE = float(C) ** -0.5
WSCALE = 16.0    # fp8 pre-scale on merged weights (power of 2, exact)
EXP_BIAS = -2.0  # constant max-proxy; cancels in the softmax ratio
DR = mybir.MatmulPerfMode.DoubleRow

N_CORES = 8


def _emit_fp8(ctx: ExitStack, tc: tile.TileContext, bias0: bool):
    nc = tc.nc
    x_d = nc.declare_dram_parameter("x", [C, N], F16, isOutput=False)
    wmk_d = nc.declare_dram_parameter("wmk", [P, 2, 2, C], F8, isOutput=False)
    wuv_d = nc.declare_dram_parameter("wuv", [P, 2, 2, C], F8, isOutput=False)
    if not bias0:
        bp2_d = nc.declare_dram_parameter("bp2", [C], F32, isOutput=False)
    gamma_d = nc.declare_dram_parameter("gamma", [C], F32, isOutput=False)
    beta_d = nc.declare_dram_parameter("beta", [C], F32, isOutput=False)
    mask_d = nc.declare_dram_parameter("gmask", [P, GPT], F32, isOutput=False)
    expand_d = nc.declare_dram_parameter("gexpand", [GPT, P], F32, isOutput=False)
    out_d = nc.declare_dram_parameter("out", [C, NQ], F16, isOutput=True)

    consts = ctx.enter_context(tc.tile_pool(name="consts", bufs=1))
    big = ctx.enter_context(tc.tile_pool(name="big", bufs=1))
    gn_small = ctx.enter_context(tc.tile_pool(name="gn_small", bufs=2))
    exp_pool = ctx.enter_context(tc.tile_pool(name="exp_pool", bufs=4))
    out_pool = ctx.enter_context(tc.tile_pool(name="out_pool", bufs=4))
    ps_mm = ctx.enter_context(tc.tile_pool(name="ps_mm", bufs=3, space="PSUM"))
    ps_att = ctx.enter_context(tc.tile_pool(name="ps_att", bufs=1, space="PSUM"))
    ps_s = ctx.enter_context(tc.tile_pool(name="ps_s", bufs=1, space="PSUM"))

    ident_f = mybir.ActivationFunctionType.Identity

    # ---- start the x stream immediately on the HWDGE (sync) queue; small
    # constants go via SWDGE (gpsimd), weights follow x on the sync queue ----
    xs_tiles = []
    for t in range(CT):
        xs = big.tile([P, N], F16, name=f"xs_{t}", tag=f"xs_{t}")
        # chunked so bn_stats can start before the whole tile lands
        for ch in range(4):
            nc.sync.dma_start(out=xs[:, ch * (N // 4):(ch + 1) * (N // 4)],
                              in_=x_d[t * P:(t + 1) * P,
                                      ch * (N // 4):(ch + 1) * (N // 4)])
        xs_tiles.append(xs)

    # small GN constants first -- the first GN matmul waits on mask/expand
    mask_sb = consts.tile([P, GPT], F32, name="mask_sb", tag="mask_sb")
    nc.gpsimd.dma_start(out=mask_sb, in_=mask_d[:, :])
    expand_sb = consts.tile([GPT, P], F32, name="expand_sb", tag="expand_sb")
    nc.gpsimd.dma_start(out=expand_sb, in_=expand_d[:, :])

    def load_vec(ap, nm):
        r = ap[:].rearrange("(t p) -> t p", p=P)
        tiles = []
        for t in range(CT):
            tl = consts.tile([P, 1], F32, name=f"{nm}_{t}", tag=f"{nm}_{t}")
            nc.gpsimd.dma_start(out=tl, in_=r[t][:, None])
            tiles.append(tl)
        return tiles

    gamma_sb = load_vec(gamma_d, "gamma")
    beta_sb = load_vec(beta_d, "beta")
    bp2_sb = None if bias0 else load_vec(bp2_d, "bp2")

    # merged weights (fp8, pre-packed on the host for DoubleRow)
    wmk_sb = consts.tile([P, 2, 2, C], F8, name="wmk_sb", tag="wmk_sb")
    nc.sync.dma_start(out=wmk_sb, in_=wmk_d[:, :, :, :])
    wuv_sb = consts.tile([P, 2, 2, C], F8, name="wuv_sb", tag="wuv_sb")
    nc.sync.dma_start(out=wuv_sb, in_=wuv_d[:, :, :, :])

    ones16 = consts.tile([P, 2, P], F8, name="ones16", tag="ones16")
    nc.vector.memset(ones16, WSCALE)
    expbias_sb = consts.tile([P, 1], F32, name="expbias_sb", tag="expbias_sb")
    nc.vector.memset(expbias_sb, EXP_BIAS)

    # ---- persistent big tensors (fp8) ----
    h_all = big.tile([P, CT, N], F8, name="h_all", tag="h_all")
    k_all = big.tile([P, CT, N], F8, name="k_all", tag="k_all")
    ut = big.tile([P, NJ, C], F8, name="ut", tag="ut")

    # ---- phase 1: GroupNorm ----
    # Stage-ordered across tiles so no engine FIFO head-of-line-blocks on a
    # PSUM roundtrip of an earlier tile: all DVE stats first, then the
    # per-tile reduction chains pipelined tile-major, applies last. The tiny
    # PSUM evacuations ride on GPSIMD to keep them out of the DVE FIFO.
    st_t, ms2_t, gmv_t, grs_t, cms_t, ab_t = {}, {}, {}, {}, {}, {}
    for t in range(CT):
        # per-channel mean/var via chunked bn_stats (starts as DMA chunks land)
        st = gn_small.tile([P, N // NB, 6], F32, name=f"st_{t}", tag=f"st_{t}")
        xs_c = xs_tiles[t].rearrange("p (c f) -> p c f", f=NB)
        for cchunk in range(N // NB):
            nc.vector.bn_stats(out=st[:, cchunk, :], in_=xs_c[:, cchunk, :])
        st_t[t] = st
    for t in range(CT):
        # aggregate [mean, var] straight into the matmul operand, then turn
        # col1 into E[x^2] = var + mean^2 in place
        ms2 = gn_small.tile([P, 2], F32, name=f"ms2_{t}", tag=f"ms2_{t}")
        nc.vector.bn_aggr(out=ms2, in_=st_t[t])
        msq = gn_small.tile([P, 1], F32, name=f"msq_{t}", tag=f"msq_{t}")
        nc.gpsimd.tensor_tensor(msq, ms2[:, 0:1], ms2[:, 0:1],
                                mybir.AluOpType.mult)
        nc.gpsimd.tensor_add(ms2[:, 1:2], ms2[:, 1:2], msq)
        ms2_t[t] = ms2
    for t in range(CT):
        # group-average across the 16-channel partition runs: mask matmul (fp32)
        gps = ps_mm.tile([GPT, 2], F32, name=f"gps_{t}", tag="mm")
        nc.tensor.matmul(gps, lhsT=mask_sb, rhs=ms2_t[t], start=True, stop=True)
        gmv = gn_small.tile([GPT, 2], F32, name=f"gmv_{t}", tag=f"gmv_{t}")
        nc.vector.tensor_copy(out=gmv, in_=gps)
        gmv_t[t] = gmv
    for t in range(CT):
        gmv = gmv_t[t]
        # vpe = var_g + eps ; rstd via ACT sqrt + reciprocal + one Newton step
        vpe = gn_small.tile([GPT, 1], F32, name=f"vpe_{t}", tag=f"vpe_{t}")
        nc.gpsimd.tensor_tensor(vpe, gmv[:, 0:1], gmv[:, 0:1], mybir.AluOpType.mult)
        nc.gpsimd.tensor_scalar(vpe, gmv[:, 1:2], vpe, EPS,
                                mybir.AluOpType.subtract, mybir.AluOpType.add)
        sd = gn_small.tile([GPT, 1], F32, name=f"sd_{t}", tag=f"sd_{t}")
        nc.scalar.sqrt(out=sd, in_=vpe)
        y0 = gn_small.tile([GPT, 1], F32, name=f"y0_{t}", tag=f"y0_{t}")
        nc.vector.reciprocal(out=y0, in_=sd)
        t1 = gn_small.tile([GPT, 1], F32, name=f"t1_{t}", tag=f"t1_{t}")
        nc.gpsimd.tensor_tensor(t1, y0, y0, mybir.AluOpType.mult)
        nc.gpsimd.tensor_tensor(t1, t1, vpe, mybir.AluOpType.mult)
        nc.gpsimd.tensor_scalar(t1, t1, -0.5, 1.5,
                                mybir.AluOpType.mult, mybir.AluOpType.add)
        grs = gn_small.tile([GPT, 2], F32, name=f"grs_{t}", tag=f"grs_{t}")
        nc.gpsimd.tensor_copy(out=grs[:, 0:1], in_=gmv[:, 0:1])
        nc.gpsimd.tensor_tensor(grs[:, 1:2], y0, t1, mybir.AluOpType.mult)
        grs_t[t] = grs
    for t in range(CT):
        # expand group stats back to channels: (GPT,P).T @ (GPT,2) -> (P,2)
        cps = ps_mm.tile([P, 2], F32, name=f"cps_{t}", tag="mm")
        nc.tensor.matmul(cps, lhsT=expand_sb, rhs=grs_t[t], start=True, stop=True)
        cms = gn_small.tile([P, 2], F32, name=f"cms_{t}", tag=f"cms_{t}")
        nc.vector.tensor_copy(out=cms, in_=cps)
        cms_t[t] = cms
    for t in range(CT):
        cms = cms_t[t]
        a_t = gn_small.tile([P, 1], F32, name=f"a_{t}", tag=f"a_{t}")
        nc.gpsimd.tensor_tensor(a_t, gamma_sb[t], cms[:, 1:2], mybir.AluOpType.mult)
        b_t = gn_small.tile([P, 1], F32, name=f"b_{t}", tag=f"b_{t}")
        nc.gpsimd.tensor_tensor(b_t, cms[:, 0:1], a_t, mybir.AluOpType.mult)
        nc.gpsimd.tensor_tensor(b_t, beta_sb[t], b_t, mybir.AluOpType.subtract)
        ab_t[t] = (a_t, b_t)
    for t in range(CT):
        a_t, b_t = ab_t[t]
        # h = x*A + B, cast to fp8 -- split across ACT and DVE so neither
        # engine serializes the GN critical path
        nc.scalar.activation(out=h_all[:, t, :N // 2], in_=xs_tiles[t][:, :N // 2],
                             func=ident_f, bias=b_t, scale=a_t)
        nc.vector.tensor_scalar(h_all[:, t, N // 2:], xs_tiles[t][:, N // 2:],
                                a_t, b_t,
                                mybir.AluOpType.mult, mybir.AluOpType.add)

    # ---- phase 2: k~ and u convs (fp8 DoubleRow, 256-deep per matmul) ----
    # Conv PSUM groups rotate over all 8 banks (ps_mm's 3 plus the 4
    # attention-accumulator banks and the S bank, idle during this phase).
    conv_n = 0

    def conv_psum(nm, free):
        nonlocal conv_n
        conv_n += 1
        r = conv_n % 8
        if r < 3:
            return ps_mm.tile([P, free], F32, name=nm, tag="mm")
        if r < 7:
            return ps_att.tile([P, free], F32, name=nm, tag=f"att{r - 3}")
        return ps_s.tile([P, free], F32, name=nm, tag="s")

    evac_n = 0

    def evacuate(dst, src):
        # alternate ACT / DVE so neither engine owns all PSUM drains
        nonlocal evac_n
        evac_n += 1
        if evac_n % 2 == 0:
            nc.scalar.copy(out=dst, in_=src)
        else:
            nc.vector.tensor_copy(out=dst, in_=src)

    for co in range(CT):
        for nb in range(N // NB):
            nsl = slice(nb * NB, (nb + 1) * NB)
            ps = conv_psum(f"kps_{co}_{nb}", NB)
            for p2 in range(2):
                nc.tensor.matmul(ps,
                                 lhsT=wmk_sb[:, p2, :, co * P:(co + 1) * P],
                                 rhs=h_all[:, 2 * p2:2 * p2 + 2, nsl],
                                 start=(p2 == 0), stop=(p2 == 1), perf_mode=DR)
            evacuate(k_all[:, co, nsl], ps)
    for j in range(NJ):
        jsl = slice(j * P, (j + 1) * P)
        ps = conv_psum(f"ups_{j}", C)
        for p2 in range(2):
            nc.tensor.matmul(ps,
                             lhsT=h_all[:, 2 * p2:2 * p2 + 2, jsl],
                             rhs=wuv_sb[:, p2, :, :],
                             start=(p2 == 0), stop=(p2 == 1), perf_mode=DR)
        evacuate(ut[:, j, :], ps)

    # ---- phase 3: attention + epilogue, per query block ----
    # Software-pipelined emission: scores(j+1) is emitted before att(pair)
    # so the PE never stalls on the ACT exp; the previous block's epilogue
    # tail is emitted two j-steps into the next block.
    def emit_tail(ib, att_ps, s_ps, last=False):
        rb = out_pool.tile([P, NB], F32, name=f"rb_{ib}", tag="rb", bufs=2)
        rscr = out_pool.tile([P, NB], F32, name=f"rscr_{ib}", tag="rscr", bufs=2)
        nc.vector.reciprocal_approx_accurate(out=rb, in_=s_ps, scratch=rscr)
        # column chunks so each chunk's store overlaps the next chunk's
        # arithmetic; quarters on the final block to shorten the serial tail,
        # with stores alternating across the two HWDGE queues so descriptor
        # processing doesn't serialize on the SP sequencer
        nh = 2
        for co in range(CT):
            fin = out_pool.tile([P, NB], F16, name=f"fin_{ib}_{co}", tag="fin")
            for hh in range(nh):
                hs = slice(hh * (NB // nh), (hh + 1) * (NB // nh))
                xsl = slice(ib * NB + hh * (NB // nh),
                            ib * NB + (hh + 1) * (NB // nh))
                nc.vector.tensor_tensor(fin[:, hs], att_ps[co][:, hs], rb[:, hs],
                                        mybir.AluOpType.mult)
                if not bias0:
                    nc.vector.tensor_scalar_add(fin[:, hs], fin[:, hs],
                                                bp2_sb[co])
                nc.vector.tensor_add(fin[:, hs], fin[:, hs],
                                     xs_tiles[co][:, xsl])
                nc.sync.dma_start(out=out_d[co * P:(co + 1) * P, xsl],
                                  in_=fin[:, hs])

    pending = None
    for ib in range(NQ // NB):
        isl = slice(ib * NB, (ib + 1) * NB)
        att_ps = [ps_att.tile([P, NB], F32, name=f"attps_{ib}_{c}", tag=f"att{c}")
                  for c in range(CT)]
        s_ps = ps_s.tile([P, NB], F32, name=f"sps_{ib}", tag="s")
        expair = None
        for j in range(NJ):
            sc = ps_mm.tile([P, NB], F32, name=f"sc_{ib}_{j}", tag="mm")
            for p2 in range(2):
                nc.tensor.matmul(sc,
                                 lhsT=k_all[:, 2 * p2:2 * p2 + 2,
                                            j * P:(j + 1) * P],
                                 rhs=h_all[:, 2 * p2:2 * p2 + 2, isl],
                                 start=(p2 == 0), stop=(p2 == 1), perf_mode=DR)
            if j % 2 == 0:
                expair = exp_pool.tile([P, 2, NB], F8, name=f"ex_{ib}_{j}",
                                       tag="exp")
            nc.scalar.activation(out=expair[:, j % 2, :], in_=sc,
                                 func=mybir.ActivationFunctionType.Exp,
                                 bias=expbias_sb, scale=SCALE / WSCALE)
            if pending is not None and j == 1:
                # previous block's epilogue slots in here, before this
                # block's first att matmul reuses the accumulator banks
                emit_tail(*pending)
                pending = None
            if j % 2 == 1:
                jp = j - 1
                # softmax denominator: all-16s stationary reduces over the
                # 256 key positions and broadcasts S to all 128 partitions.
                # On the final pair S goes first so the reciprocal overlaps
                # the remaining att matmuls.
                def emit_s():
                    nc.tensor.matmul(s_ps, lhsT=ones16, rhs=expair,
                                     start=(jp == 0), stop=(jp == NJ - 2),
                                     perf_mode=DR)
                if jp == NJ - 2:
                    emit_s()
                for c in range(CT):
                    nc.tensor.matmul(att_ps[c],
                                     lhsT=ut[:, jp:jp + 2, c * P:(c + 1) * P],
                                     rhs=expair,
                                     start=(jp == 0), stop=(jp == NJ - 2),
                                     perf_mode=DR)
                if jp != NJ - 2:
                    emit_s()
        pending = (ib, att_ps, s_ps)
    emit_tail(*pending, last=True)


def _emit_general(ctx: ExitStack, tc: tile.TileContext):
    """Fallback: separate q/k convs with biases, fp16 operands (baseline)."""
    nc = tc.nc
    x_d = nc.declare_dram_parameter("x", [C, N], F32, isOutput=False)
    wqT_d = nc.declare_dram_parameter("wqT", [C, C], F16, isOutput=False)
    wkT_d = nc.declare_dram_parameter("wkT", [C, C], F16, isOutput=False)
    wvT_d = nc.declare_dram_parameter("wvT", [C, C], F16, isOutput=False)
    wpT_d = nc.declare_dram_parameter("wpT", [C, C], F16, isOutput=False)
    bq_d = nc.declare_dram_parameter("bq", [C], F32, isOutput=False)
    bk_d = nc.declare_dram_parameter("bk", [C], F32, isOutput=False)
    bp2_d = nc.declare_dram_parameter("bp2", [C], F32, isOutput=False)
    gamma_d = nc.declare_dram_parameter("gamma", [C], F32, isOutput=False)
    beta_d = nc.declare_dram_parameter("beta", [C], F32, isOutput=False)
    mask_d = nc.declare_dram_parameter("gmask", [P, GPT], F32, isOutput=False)
    expand_d = nc.declare_dram_parameter("gexpand", [GPT, P], F32, isOutput=False)
    out_d = nc.declare_dram_parameter("out", [C, NQ], F32, isOutput=True)

    consts = ctx.enter_context(tc.tile_pool(name="consts", bufs=1))
    big = ctx.enter_context(tc.tile_pool(name="big", bufs=1))
    stage = ctx.enter_context(tc.tile_pool(name="stage", bufs=2))
    gn_small = ctx.enter_context(tc.tile_pool(name="gn_small", bufs=2))
    exp_pool = ctx.enter_context(tc.tile_pool(name="exp_pool", bufs=4))
    att_sb_pool = ctx.enter_context(tc.tile_pool(name="att_sb_pool", bufs=2))
    out_pool = ctx.enter_context(tc.tile_pool(name="out_pool", bufs=4))
    ps_mm = ctx.enter_context(tc.tile_pool(name="ps_mm", bufs=4, space="PSUM"))
    ps_att = ctx.enter_context(tc.tile_pool(name="ps_att", bufs=1, space="PSUM"))

    ident_f = mybir.ActivationFunctionType.Identity

    xs_tiles = []
    for t in range(CT):
        xs = stage.tile([P, N], F32, name=f"xs_{t}", tag="xs")
        for ch in range(4):
            nc.sync.dma_start(out=xs[:, ch * (N // 4):(ch + 1) * (N // 4)],
                              in_=x_d[t * P:(t + 1) * P,
                                      ch * (N // 4):(ch + 1) * (N // 4)])
        xs_tiles.append(xs)

    mask_sb = consts.tile([P, GPT], F32, name="mask_sb", tag="mask_sb")
    nc.gpsimd.dma_start(out=mask_sb, in_=mask_d[:, :])
    expand_sb = consts.tile([GPT, P], F32, name="expand_sb", tag="expand_sb")
    nc.gpsimd.dma_start(out=expand_sb, in_=expand_d[:, :])

    def load_vec(ap, nm):
        r = ap[:].rearrange("(t p) -> t p", p=P)
        tiles = []
        for t in range(CT):
            tl = consts.tile([P, 1], F32, name=f"{nm}_{t}", tag=f"{nm}_{t}")
            nc.gpsimd.dma_start(out=tl, in_=r[t][:, None])
            tiles.append(tl)
        return tiles

    gamma_sb = load_vec(gamma_d, "gamma")
    beta_sb = load_vec(beta_d, "beta")
    bq_sb = load_vec(bq_d, "bq")
    bk_sb = load_vec(bk_d, "bk")
    bp2_sb = load_vec(bp2_d, "bp2")

    w_sb = {}
    w_order = (("k", wkT_d), ("v", wvT_d), ("q", wqT_d), ("p", wpT_d))
    for wname, w_ap in w_order:
        for t in range(CT):
            tl = consts.tile([P, C], F16, name=f"w{wname}_{t}", tag=f"w{wname}_{t}")
            nc.sync.dma_start(out=tl, in_=w_ap[t * P:(t + 1) * P, :])
            w_sb[wname, t] = tl
    ones32 = consts.tile([P, P], F32, name="ones32", tag="ones32")
    nc.vector.memset(ones32, 1.0)
    expbias_sb = consts.tile([P, 1], F32, name="expbias_sb", tag="expbias_sb")
    nc.vector.memset(expbias_sb, -4.0)

    h_sb = [big.tile([P, N], F16, name=f"h_{t}", tag=f"h_{t}") for t in range(CT)]
    k_sb = [big.tile([P, N], F16, name=f"k_{t}", tag=f"k_{t}") for t in range(CT)]
    q_sb = [big.tile([P, NQ], F16, name=f"q_{t}", tag=f"q_{t}") for t in range(CT)]
    vt_sb = big.tile([P, NJ, C], F16, name="vt_sb", tag="vt_sb")

    for t in range(CT):
        xs = xs_tiles[t]
        st = gn_small.tile([P, N // NB, 6], F32, name=f"st_{t}", tag="st")
        xs_c = xs.rearrange("p (c f) -> p c f", f=NB)
        for cchunk in range(N // NB):
            nc.vector.bn_stats(out=st[:, cchunk, :], in_=xs_c[:, cchunk, :])
        ms2 = gn_small.tile([P, 2], F32, name=f"ms2_{t}", tag="ms2")
        nc.vector.bn_aggr(out=ms2, in_=st)
        msq = gn_small.tile([P, 1], F32, name=f"msq_{t}", tag="msq")
        nc.gpsimd.tensor_tensor(msq, ms2[:, 0:1], ms2[:, 0:1],
                                mybir.AluOpType.mult)
        nc.gpsimd.tensor_add(ms2[:, 1:2], ms2[:, 1:2], msq)
        gps = ps_mm.tile([GPT, 2], F32, name=f"gps_{t}", tag="mm")
        nc.tensor.matmul(gps, lhsT=mask_sb, rhs=ms2, start=True, stop=True)
        gmv = gn_small.tile([GPT, 2], F32, name=f"gmv_{t}", tag="gmv")
        nc.vector.tensor_copy(out=gmv, in_=gps)
        vpe = gn_small.tile([GPT, 1], F32, name=f"vpe_{t}", tag="vpe")
        nc.gpsimd.tensor_tensor(vpe, gmv[:, 0:1], gmv[:, 0:1], mybir.AluOpType.mult)
        nc.gpsimd.tensor_scalar(vpe, gmv[:, 1:2], vpe, EPS,
                                mybir.AluOpType.subtract, mybir.AluOpType.add)
        sd = gn_small.tile([GPT, 1], F32, name=f"sd_{t}", tag="sd")
        nc.scalar.sqrt(out=sd, in_=vpe)
        y0 = gn_small.tile([GPT, 1], F32, name=f"y0_{t}", tag="y0")
        nc.vector.reciprocal(out=y0, in_=sd)
        t1 = gn_small.tile([GPT, 1], F32, name=f"t1_{t}", tag="t1")
        nc.gpsimd.tensor_tensor(t1, y0, y0, mybir.AluOpType.mult)
        nc.gpsimd.tensor_tensor(t1, t1, vpe, mybir.AluOpType.mult)
        nc.gpsimd.tensor_scalar(t1, t1, -0.5, 1.5,
                                mybir.AluOpType.mult, mybir.AluOpType.add)
        grs = gn_small.tile([GPT, 2], F32, name=f"grs_{t}", tag="grs")
        nc.gpsimd.tensor_copy(out=grs[:, 0:1], in_=gmv[:, 0:1])
        nc.gpsimd.tensor_tensor(grs[:, 1:2], y0, t1, mybir.AluOpType.mult)
        cps = ps_mm.tile([P, 2], F32, name=f"cps_{t}", tag="mm")
        nc.tensor.matmul(cps, lhsT=expand_sb, rhs=grs, start=True, stop=True)
        cms = gn_small.tile([P, 2], F32, name=f"cms_{t}", tag="cms")
        nc.vector.tensor_copy(out=cms, in_=cps)
        a_t = gn_small.tile([P, 1], F32, name=f"a_{t}", tag="a")
        nc.gpsimd.tensor_tensor(a_t, gamma_sb[t], cms[:, 1:2], mybir.AluOpType.mult)
        b_t = gn_small.tile([P, 1], F32, name=f"b_{t}", tag="b")
        nc.gpsimd.tensor_tensor(b_t, cms[:, 0:1], a_t, mybir.AluOpType.mult)
        nc.gpsimd.tensor_tensor(b_t, beta_sb[t], b_t, mybir.AluOpType.subtract)
        nc.scalar.activation(out=h_sb[t][:, :N // 2], in_=xs[:, :N // 2],
                             func=ident_f, bias=b_t, scale=a_t)
        nc.vector.tensor_scalar(h_sb[t][:, N // 2:], xs[:, N // 2:], a_t, b_t,
                                mybir.AluOpType.mult, mybir.AluOpType.add)

    conv_n = 0

    def conv_psum(nm, free):
        nonlocal conv_n
        conv_n += 1
        if conv_n % 8 < 4:
            return ps_mm.tile([P, free], F32, name=nm, tag="mm")
        return ps_att.tile([P, free], F32, name=nm, tag=f"att{conv_n % 8 - 4}")

    for co in range(CT):
        for nb in range(N // NB):
            ps = conv_psum(f"kps_{co}_{nb}", NB)
            for ci in range(CT):
                nc.tensor.matmul(ps, lhsT=w_sb["k", ci][:, co * P:(co + 1) * P],
                                 rhs=h_sb[ci][:, nb * NB:(nb + 1) * NB],
                                 start=(ci == 0), stop=(ci == CT - 1))
            nc.scalar.activation(out=k_sb[co][:, nb * NB:(nb + 1) * NB],
                                 in_=ps, func=ident_f, bias=bk_sb[co], scale=1.0)
    for co in range(CT):
        for nb in range(NQ // NB):
            ps = conv_psum(f"qps_{co}_{nb}", NB)
            for ci in range(CT):
                nc.tensor.matmul(ps,
                                 lhsT=w_sb["q", ci][:, co * P:(co + 1) * P],
                                 rhs=h_sb[ci][:, nb * NB:(nb + 1) * NB],
                                 start=(ci == 0), stop=(ci == CT - 1))
            nc.scalar.activation(out=q_sb[co][:, nb * NB:(nb + 1) * NB],
                                 in_=ps, func=ident_f, bias=bq_sb[co],
                                 scale=1.0)
    for j in range(NJ):
        ps = conv_psum(f"vps_{j}", C)
        for ci in range(CT):
            nc.tensor.matmul(ps, lhsT=h_sb[ci][:, j * P:(j + 1) * P],
                             rhs=w_sb["v", ci],
                             start=(ci == 0), stop=(ci == CT - 1))
        nc.scalar.copy(out=vt_sb[:, j, :], in_=ps)

    def emit_tail(ib, att_ps, sacc):
        isl = slice(ib * NB, (ib + 1) * NB)
        sps = ps_mm.tile([P, NB], F32, name=f"sps_{ib}", tag="mm")
        nc.tensor.matmul(sps, lhsT=ones32, rhs=sacc, start=True, stop=True)
        rb = out_pool.tile([P, NB], F32, name=f"rb_{ib}", tag="rb", bufs=2)
        rscr = out_pool.tile([P, NB], F32, name=f"rscr_{ib}", tag="rscr", bufs=2)
        nc.vector.reciprocal_approx_accurate(out=rb, in_=sps, scratch=rscr)
        att_sb = []
        for c in range(CT):
            asb = att_sb_pool.tile([P, NB], F16, name=f"attsb_{ib}_{c}",
                                   tag=f"asb{c}")
            nc.scalar.copy(out=asb, in_=att_ps[c])
            att_sb.append(asb)
        for co in range(CT):
            xres = out_pool.tile([P, NB], F32, name=f"xres_{ib}_{co}", tag="xres")
            nc.gpsimd.dma_start(out=xres, in_=x_d[co * P:(co + 1) * P, isl])
            pp = ps_mm.tile([P, NB], F32, name=f"pp_{ib}_{co}", tag="mm")
            for ci in range(CT):
                nc.tensor.matmul(pp, lhsT=w_sb["p", ci][:, co * P:(co + 1) * P],
                                 rhs=att_sb[ci],
                                 start=(ci == 0), stop=(ci == CT - 1))
            fin = out_pool.tile([P, NB], F32, name=f"fin_{ib}_{co}", tag="fin")
            for hh in range(2):
                hs = slice(hh * (NB // 2), (hh + 1) * (NB // 2))
                nc.vector.tensor_tensor(fin[:, hs], pp[:, hs], rb[:, hs],
                                        mybir.AluOpType.mult)
                nc.vector.tensor_scalar_add(fin[:, hs], fin[:, hs], bp2_sb[co])
                nc.vector.tensor_add(fin[:, hs], fin[:, hs], xres[:, hs])
                nc.sync.dma_start(
                    out=out_d[co * P:(co + 1) * P,
                              ib * NB + hh * (NB // 2):
                              ib * NB + (hh + 1) * (NB // 2)],
                    in_=fin[:, hs])

    pending = None
    for ib in range(NQ // NB):
        isl = slice(ib * NB, (ib + 1) * NB)
        att_ps = [ps_att.tile([P, NB], F32, name=f"attps_{ib}_{c}", tag=f"att{c}")
                  for c in range(CT)]
        sacc = out_pool.tile([P, NB], F32, name=f"sacc_{ib}", tag="sacc", bufs=2)
        ex_tiles = {}
        for j in range(NJ + 1):
            if j < NJ:
                sc = ps_mm.tile([P, NB], F32, name=f"sc_{ib}_{j}", tag="mm")
                for ci in range(CT):
                    nc.tensor.matmul(sc, lhsT=k_sb[ci][:, j * P:(j + 1) * P],
                                     rhs=q_sb[ci][:, isl],
                                     start=(ci == 0), stop=(ci == CT - 1))
                ex = exp_pool.tile([P, NB], F16, name=f"ex_{ib}_{j}", tag="exp")
                nc.scalar.activation(out=ex, in_=sc,
                                     func=mybir.ActivationFunctionType.Exp,
                                     bias=expbias_sb, scale=SCALE)
                ex_tiles[j] = ex
            if pending is not None and j == 1:
                emit_tail(*pending)
                pending = None
            if j >= 1:
                jp = j - 1
                ex = ex_tiles.pop(jp)
                for c in range(CT):
                    nc.tensor.matmul(att_ps[c],
                                     lhsT=vt_sb[:, jp, c * P:(c + 1) * P],
                                     rhs=ex, start=(jp == 0), stop=(jp == NJ - 1))
                if jp == 0:
                    nc.vector.tensor_copy(out=sacc, in_=ex)
                else:
                    nc.vector.tensor_add(sacc, sacc, ex)
        pending = (ib, att_ps, sacc)
    emit_tail(*pending)


_CACHED = {}


def _build(merged=True, bias0=True):
    key = ("fp8", bias0) if merged else ("gen",)
    if key not in _CACHED:
        nc = bacc.Bacc()
        with tile.TileContext(nc) as tc, ExitStack() as ctx:
            if merged:
                _emit_fp8(ctx, tc, bias0)
            else:
                _emit_general(ctx, tc)
        nc.finalize()
        _CACHED[key] = nc
    return _CACHED[key]


def _to_f8(a):
    return np.asarray(np.clip(a, -240.0, 240.0),
                      dtype=ml_dtypes.float8_e4m3)


def _pack_dr(wT):
    # [C_in, C_out] -> [128, 2(pair), 2(elem), C_out] for DoubleRow lhsT/rhs
    return np.ascontiguousarray(
        np.transpose(wT.reshape(2, 2, P, C), (2, 0, 1, 3)))


def _host_inputs(x, norm_gamma, norm_beta, Wq, bq, Wk, bk, Wv, bv, Wp, bp,
                 merged=None):
    if merged is None:
        merged = (not np.any(np.asarray(bq))) and (not np.any(np.asarray(bk)))
    bp2 = (np.asarray(Wp, np.float64) @ np.asarray(bv, np.float64)
           + np.asarray(bp, np.float64)).astype(np.float32)
    common = {
        "gamma": np.asarray(norm_gamma, np.float32),
        "beta": np.asarray(norm_beta, np.float32),
        "gmask": ((np.arange(P)[:, None] // GS == np.arange(GPT)[None, :])
                  .astype(np.float32) / GS),
        "gexpand": (np.arange(GPT)[:, None] == np.arange(P)[None, :] // GS)
                   .astype(np.float32),
    }
    if merged:
        bias0 = not np.any(bp2)
        wmT = (np.asarray(Wk, np.float64).T @ np.asarray(Wq, np.float64))
        wuT = (np.asarray(Wp, np.float64) @ np.asarray(Wv, np.float64)).T
        common["wmk"] = _pack_dr(_to_f8(wmT * WSCALE))
        common["wuv"] = _pack_dr(_to_f8(wuT * WSCALE))
        if not bias0:
            common["bp2"] = bp2
    else:
        common["wqT"] = np.ascontiguousarray(
            np.asarray(Wq, np.float32).T).astype(np.float16)
        common["wkT"] = np.ascontiguousarray(
            np.asarray(Wk, np.float32).T).astype(np.float16)
        common["wvT"] = np.ascontiguousarray(
            np.asarray(Wv, np.float32).T).astype(np.float16)
        common["wpT"] = np.ascontiguousarray(
            np.asarray(Wp, np.float32).T).astype(np.float16)
        common["bq"] = np.asarray(bq, np.float32)
        common["bk"] = np.asarray(bk, np.float32)
        common["bp2"] = bp2
    xf = np.asarray(x, np.float32).reshape(4, C, N)
    if merged:
        xf = xf.astype(np.float16)
    in_maps = []
    for core in range(N_CORES):
        bi, qh = core // 2, core % 2
        xc = np.ascontiguousarray(np.roll(xf[bi], -qh * NQ, axis=1))
        in_maps.append({"x": xc, **common})
    return in_maps


def kernel(x, norm_gamma, norm_beta, Wq, bq, Wk, bk, Wv, bv, Wp, bp):
    x = np.asarray(x, np.float32)
    b, c, hh, ww = x.shape
    assert (b, c, hh * ww) == (4, C, N)
    merged = (not np.any(np.asarray(bq))) and (not np.any(np.asarray(bk)))
    bp2 = np.asarray(Wp, np.float64) @ np.asarray(bv, np.float64) \
        + np.asarray(bp, np.float64)
    bias0 = not np.any(bp2)
    nc = _build(merged, bias0)
    in_maps = _host_inputs(x, norm_gamma, norm_beta,
                           Wq, bq, Wk, bk, Wv, bv, Wp, bp, merged=merged)
    res = run_bass_kernel_spmd(nc, in_maps, core_ids=list(range(N_CORES)))
    y = np.empty((4, C, N), np.float32)
    for core in range(N_CORES):
        bi, qh = core // 2, core % 2
        y[bi][:, qh * NQ:(qh + 1) * NQ] = \
            res.results[core]["out"].astype(np.float32)
    return y.reshape(b, c, hh, ww)


# revision 17
# speedup vs baseline: 1.0140x; 1.0140x over previous
"""AttnBlock (GroupNorm + spatial self-attention + proj + residual) on 8 TRN2 cores.

Problem shapes (hardcoded): x (4, 512, 64, 64) fp32, 1x1-conv weights (512, 512).

Sharding: 8 cores = (batch b in 0..3) x (query half qh in 0..1). Attention is
permutation-invariant over key positions, so each core receives its batch's
x rotated along the flattened spatial axis so that its own 2048 query
positions are always columns 0:2048 -- the compiled NEFF is identical on all
cores (pure SPMD, no collectives). Each core computes GroupNorm, k~ and u for
all 4096 positions, attention for its 2048 queries and the residual,
producing a (512, 2048) fp32 shard.

Fast path (bq == bk == 0, true for this problem): all heavy matmuls run as
fp8e4 DoubleRow (256-deep contraction per instruction, ~2x PE throughput).
Two host-side weight merges remove two of the four convs:
  k~ = (Wq^T Wk) h   so scores = k~^T h (q conv gone; softmax is invariant
                     to the per-query constant)
  u  = (Wp Wv) h + (Wp bv)  since the proj conv commutes with the attention
                     average: out = x + (sum_j a_ij u_j)/S_i + bp2
Merged weights are pre-scaled by 16 to keep their entries out of the fp8e4
subnormal range; the 1/16 folds into the exp scale and into the softmax
denominator (whose all-ones matmul stationary becomes all-16s). The
denominator itself is accumulated on the PE by an extra DoubleRow matmul per
key pair whose [128, 2, 128] stationary is constant 16.0 -- the matmul both
reduces over partitions and broadcasts S to all 128 output partitions, so
the epilogue is just two DVE ops per tile: fin = att_psum * (1/S) + x, with
x reused from the SBUF-resident staging tiles (no residual re-read from HBM).

Numerics: GroupNorm/softmax statistics in fp32, fp32 PSUM accumulation
everywhere; fp8 only on matmul operands. Measured end-to-end rel err ~4e-3.

A general fallback (separate q/k convs, fp16 operands, nonzero biases) is
kept and selected automatically when bq/bk are nonzero.
"""

from contextlib import ExitStack

import ml_dtypes
import numpy as np

import concourse.bacc as bacc
import concourse.mybir as mybir
import concourse.tile as tile
from concourse.bass_utils import run_bass_kernel_spmd

F32 = mybir.dt.float32
F16 = mybir.dt.float16
F8 = mybir.dt.float8e4

C = 512          # channels
N = 4096         # spatial positions (64*64)
NQ = 2048        # query positions per core
P = 128          # partitions
CT = C // P      # 4 channel tiles
NB = 512         # matmul free-dim block
NJ = N // P      # 32 key tiles
G = 32           # groups
GS = C // G      # 16 channels per group
GPT = P // GS    # 8 groups per channel tile
EPS = 1e-6
SCALE = float(C) ** -0.5
WSCALE = 16.0    # fp8 pre-scale on merged weights (power of 2, exact)
EXP_BIAS = -2.0  # constant max-proxy; cancels in the softmax ratio
DR = mybir.MatmulPerfMode.DoubleRow

N_CORES = 8


def _emit_fp8(ctx: ExitStack, tc: tile.TileContext, bias0: bool):
    nc = tc.nc
    x_d = nc.declare_dram_parameter("x", [C, N], F16, isOutput=False)
    wmk_d = nc.declare_dram_parameter("wmk", [P, 2, 2, C], F8, isOutput=False)
    wuv_d = nc.declare_dram_parameter("wuv", [P, 2, 2, C], F8, isOutput=False)
    if not bias0:
        bp2_d = nc.declare_dram_parameter("bp2", [C], F32, isOutput=False)
    gamma_d = nc.declare_dram_parameter("gamma", [C], F32, isOutput=False)
    beta_d = nc.declare_dram_parameter("beta", [C], F32, isOutput=False)
    mask_d = nc.declare_dram_parameter("gmask", [P, GPT], F32, isOutput=False)
    expand_d = nc.declare_dram_parameter("gexpand", [GPT, P], F32, isOutput=False)
    out_d = nc.declare_dram_parameter("out", [C, NQ], F16, isOutput=True)

    consts = ctx.enter_context(tc.tile_pool(name="consts", bufs=1))
    big = ctx.enter_context(tc.tile_pool(name="big", bufs=1))
    gn_small = ctx.enter_context(tc.tile_pool(name="gn_small", bufs=2))
    exp_pool = ctx.enter_context(tc.tile_pool(name="exp_pool", bufs=4))
    out_pool = ctx.enter_context(tc.tile_pool(name="out_pool", bufs=4))
    ps_mm = ctx.enter_context(tc.tile_pool(name="ps_mm", bufs=3, space="PSUM"))
    ps_att = ctx.enter_context(tc.tile_pool(name="ps_att", bufs=1, space="PSUM"))
    ps_s = ctx.enter_context(tc.tile_pool(name="ps_s", bufs=1, space="PSUM"))

    ident_f = mybir.ActivationFunctionType.Identity

    # ---- start the x stream immediately on the HWDGE (sync) queue; small
    # constants go via SWDGE (gpsimd), weights follow x on the sync queue ----
    xs_tiles = []
    for t in range(CT):
        xs = big.tile([P, N], F16, name=f"xs_{t}", tag=f"xs_{t}")
        # chunked so bn_stats can start before the whole tile lands; 2048
        # columns = 4KB contiguous per partition row keeps DMA at full rate
        for ch in range(2):
            nc.sync.dma_start(out=xs[:, ch * (N // 2):(ch + 1) * (N // 2)],
                              in_=x_d[t * P:(t + 1) * P,
                                      ch * (N // 2):(ch + 1) * (N // 2)])
        xs_tiles.append(xs)

    # small GN constants first -- the first GN matmul waits on mask/expand
    mask_sb = consts.tile([P, GPT], F32, name="mask_sb", tag="mask_sb")
    nc.gpsimd.dma_start(out=mask_sb, in_=mask_d[:, :])
    expand_sb = consts.tile([GPT, P], F32, name="expand_sb", tag="expand_sb")
    nc.gpsimd.dma_start(out=expand_sb, in_=expand_d[:, :])

    def load_vec(ap, nm):
        r = ap[:].rearrange("(t p) -> t p", p=P)
        tiles = []
        for t in range(CT):
            tl = consts.tile([P, 1], F32, name=f"{nm}_{t}", tag=f"{nm}_{t}")
            nc.gpsimd.dma_start(out=tl, in_=r[t][:, None])
            tiles.append(tl)
        return tiles

    gamma_sb = load_vec(gamma_d, "gamma")
    beta_sb = load_vec(beta_d, "beta")
    bp2_sb = None if bias0 else load_vec(bp2_d, "bp2")

    # merged weights (fp8, pre-packed on the host for DoubleRow)
    wmk_sb = consts.tile([P, 2, 2, C], F8, name="wmk_sb", tag="wmk_sb")
    nc.sync.dma_start(out=wmk_sb, in_=wmk_d[:, :, :, :])
    wuv_sb = consts.tile([P, 2, 2, C], F8, name="wuv_sb", tag="wuv_sb")
    nc.sync.dma_start(out=wuv_sb, in_=wuv_d[:, :, :, :])

    ones16 = consts.tile([P, 2, P], F8, name="ones16", tag="ones16")
    nc.vector.memset(ones16, WSCALE)
    expbias_sb = consts.tile([P, 1], F32, name="expbias_sb", tag="expbias_sb")
    nc.vector.memset(expbias_sb, EXP_BIAS)

    # ---- persistent big tensors (fp8) ----
    h_all = big.tile([P, CT, N], F8, name="h_all", tag="h_all")
    k_all = big.tile([P, CT, N], F8, name="k_all", tag="k_all")
    ut = big.tile([P, NJ, C], F8, name="ut", tag="ut")
    # epilogue staging: collect fin blocks per channel tile and store with
    # wide (2KB-row) DMAs every second block
    fin_sb = [big.tile([P, NQ], F16, name=f"fin_{co}", tag=f"fin_{co}")
              for co in range(CT)]

    # ---- phase 1: GroupNorm ----
    # Stage-ordered across tiles so no engine FIFO head-of-line-blocks on a
    # PSUM roundtrip of an earlier tile: all DVE stats first, then the
    # per-tile reduction chains pipelined tile-major, applies last. The tiny
    # PSUM evacuations ride on GPSIMD to keep them out of the DVE FIFO.
    st_t, ms2_t, gmv_t, grs_t, cms_t, ab_t = {}, {}, {}, {}, {}, {}
    for t in range(CT):
        # per-channel mean/var via chunked bn_stats (starts as DMA chunks land)
        st = gn_small.tile([P, N // NB, 6], F32, name=f"st_{t}", tag=f"st_{t}")
        xs_c = xs_tiles[t].rearrange("p (c f) -> p c f", f=NB)
        for cchunk in range(N // NB):
            nc.vector.bn_stats(out=st[:, cchunk, :], in_=xs_c[:, cchunk, :])
        st_t[t] = st
    for t in range(CT):
        # aggregate [mean, var] straight into the matmul operand, then turn
        # col1 into E[x^2] = var + mean^2 in place
        ms2 = gn_small.tile([P, 2], F32, name=f"ms2_{t}", tag=f"ms2_{t}")
        nc.vector.bn_aggr(out=ms2, in_=st_t[t])
        msq = gn_small.tile([P, 1], F32, name=f"msq_{t}", tag=f"msq_{t}")
        nc.gpsimd.tensor_tensor(msq, ms2[:, 0:1], ms2[:, 0:1],
                                mybir.AluOpType.mult)
        nc.gpsimd.tensor_add(ms2[:, 1:2], ms2[:, 1:2], msq)
        ms2_t[t] = ms2
    for t in range(CT):
        # group-average across the 16-channel partition runs: mask matmul (fp32)
        gps = ps_mm.tile([GPT, 2], F32, name=f"gps_{t}", tag="mm")
        nc.tensor.matmul(gps, lhsT=mask_sb, rhs=ms2_t[t], start=True, stop=True)
        gmv = gn_small.tile([GPT, 2], F32, name=f"gmv_{t}", tag=f"gmv_{t}")
        nc.vector.tensor_copy(out=gmv, in_=gps)
        gmv_t[t] = gmv
    for t in range(CT):
        gmv = gmv_t[t]
        # vpe = var_g + eps ; rstd via ACT sqrt + reciprocal + one Newton step
        vpe = gn_small.tile([GPT, 1], F32, name=f"vpe_{t}", tag=f"vpe_{t}")
        nc.gpsimd.tensor_tensor(vpe, gmv[:, 0:1], gmv[:, 0:1], mybir.AluOpType.mult)
        nc.gpsimd.tensor_scalar(vpe, gmv[:, 1:2], vpe, EPS,
                                mybir.AluOpType.subtract, mybir.AluOpType.add)
        sd = gn_small.tile([GPT, 1], F32, name=f"sd_{t}", tag=f"sd_{t}")
        nc.scalar.sqrt(out=sd, in_=vpe)
        y0 = gn_small.tile([GPT, 1], F32, name=f"y0_{t}", tag=f"y0_{t}")
        nc.vector.reciprocal(out=y0, in_=sd)
        t1 = gn_small.tile([GPT, 1], F32, name=f"t1_{t}", tag=f"t1_{t}")
        nc.gpsimd.tensor_tensor(t1, y0, y0, mybir.AluOpType.mult)
        nc.gpsimd.tensor_tensor(t1, t1, vpe, mybir.AluOpType.mult)
        nc.gpsimd.tensor_scalar(t1, t1, -0.5, 1.5,
                                mybir.AluOpType.mult, mybir.AluOpType.add)
        grs = gn_small.tile([GPT, 2], F32, name=f"grs_{t}", tag=f"grs_{t}")
        nc.gpsimd.tensor_copy(out=grs[:, 0:1], in_=gmv[:, 0:1])
        nc.gpsimd.tensor_tensor(grs[:, 1:2], y0, t1, mybir.AluOpType.mult)
        grs_t[t] = grs
    for t in range(CT):
        # expand group stats back to channels: (GPT,P).T @ (GPT,2) -> (P,2)
        cps = ps_mm.tile([P, 2], F32, name=f"cps_{t}", tag="mm")
        nc.tensor.matmul(cps, lhsT=expand_sb, rhs=grs_t[t], start=True, stop=True)
        cms = gn_small.tile([P, 2], F32, name=f"cms_{t}", tag=f"cms_{t}")
        nc.vector.tensor_copy(out=cms, in_=cps)
        cms_t[t] = cms
    for t in range(CT):
        cms = cms_t[t]
        a_t = gn_small.tile([P, 1], F32, name=f"a_{t}", tag=f"a_{t}")
        nc.gpsimd.tensor_tensor(a_t, gamma_sb[t], cms[:, 1:2], mybir.AluOpType.mult)
        b_t = gn_small.tile([P, 1], F32, name=f"b_{t}", tag=f"b_{t}")
        nc.gpsimd.tensor_tensor(b_t, cms[:, 0:1], a_t, mybir.AluOpType.mult)
        nc.gpsimd.tensor_tensor(b_t, beta_sb[t], b_t, mybir.AluOpType.subtract)
        ab_t[t] = (a_t, b_t)
    for t in range(CT):
        a_t, b_t = ab_t[t]
        # h = x*A + B, cast to fp8 -- split across ACT and DVE so neither
        # engine serializes the GN critical path
        nc.scalar.activation(out=h_all[:, t, :N // 2], in_=xs_tiles[t][:, :N // 2],
                             func=ident_f, bias=b_t, scale=a_t)
        nc.vector.tensor_scalar(h_all[:, t, N // 2:], xs_tiles[t][:, N // 2:],
                                a_t, b_t,
                                mybir.AluOpType.mult, mybir.AluOpType.add)

    # ---- phase 2: k~ and u convs (fp8 DoubleRow, 256-deep per matmul) ----
    # Conv PSUM groups rotate over all 8 banks (ps_mm's 3 plus the 4
    # attention-accumulator banks and the S bank, idle during this phase).
    conv_n = 0

    def conv_psum(nm, free):
        nonlocal conv_n
        conv_n += 1
        r = conv_n % 8
        if r < 3:
            return ps_mm.tile([P, free], F32, name=nm, tag="mm")
        if r < 7:
            return ps_att.tile([P, free], F32, name=nm, tag=f"att{r - 3}")
        return ps_s.tile([P, free], F32, name=nm, tag="s")

    evac_n = 0

    def evacuate(dst, src):
        # alternate ACT / DVE so neither engine owns all PSUM drains
        nonlocal evac_n
        evac_n += 1
        if evac_n % 2 == 0:
            nc.scalar.copy(out=dst, in_=src)
        else:
            nc.vector.tensor_copy(out=dst, in_=src)

    for co in range(CT):
        for nb in range(N // NB):
            nsl = slice(nb * NB, (nb + 1) * NB)
            ps = conv_psum(f"kps_{co}_{nb}", NB)
            for p2 in range(2):
                nc.tensor.matmul(ps,
                                 lhsT=wmk_sb[:, p2, :, co * P:(co + 1) * P],
                                 rhs=h_all[:, 2 * p2:2 * p2 + 2, nsl],
                                 start=(p2 == 0), stop=(p2 == 1), perf_mode=DR)
            evacuate(k_all[:, co, nsl], ps)
    for j in range(NJ):
        jsl = slice(j * P, (j + 1) * P)
        ps = conv_psum(f"ups_{j}", C)
        for p2 in range(2):
            nc.tensor.matmul(ps,
                             lhsT=h_all[:, 2 * p2:2 * p2 + 2, jsl],
                             rhs=wuv_sb[:, p2, :, :],
                             start=(p2 == 0), stop=(p2 == 1), perf_mode=DR)
        evacuate(ut[:, j, :], ps)

    # ---- phase 3: attention + epilogue, per query block ----
    # Software-pipelined emission: scores(j+1) is emitted before att(pair)
    # so the PE never stalls on the ACT exp; the previous block's epilogue
    # tail is emitted two j-steps into the next block.
    def emit_tail(ib, att_ps, s_ps, last=False):
        rb = out_pool.tile([P, NB], F32, name=f"rb_{ib}", tag="rb", bufs=2)
        rscr = out_pool.tile([P, NB], F32, name=f"rscr_{ib}", tag="rscr", bufs=2)
        nc.vector.reciprocal_approx_accurate(out=rb, in_=s_ps, scratch=rscr)
        for co in range(CT):
            isl = slice(ib * NB, (ib + 1) * NB)
            fin = fin_sb[co]
            nc.vector.tensor_tensor(fin[:, isl], att_ps[co][:, :], rb,
                                    mybir.AluOpType.mult)
            if not bias0:
                nc.vector.tensor_scalar_add(fin[:, isl], fin[:, isl],
                                            bp2_sb[co])
            nc.vector.tensor_add(fin[:, isl], fin[:, isl],
                                 xs_tiles[co][:, isl])
            if ib % 2 == 1:
                # store two finished blocks: 1024 fp16 columns = 2KB rows
                osl = slice((ib - 1) * NB, (ib + 1) * NB)
                nc.sync.dma_start(out=out_d[co * P:(co + 1) * P, osl],
                                  in_=fin[:, osl])

    pending = None
    for ib in range(NQ // NB):
        isl = slice(ib * NB, (ib + 1) * NB)
        att_ps = [ps_att.tile([P, NB], F32, name=f"attps_{ib}_{c}", tag=f"att{c}")
                  for c in range(CT)]
        s_ps = ps_s.tile([P, NB], F32, name=f"sps_{ib}", tag="s")
        expair = None
        for j in range(NJ):
            sc = ps_mm.tile([P, NB], F32, name=f"sc_{ib}_{j}", tag="mm")
            for p2 in range(2):
                nc.tensor.matmul(sc,
                                 lhsT=k_all[:, 2 * p2:2 * p2 + 2,
                                            j * P:(j + 1) * P],
                                 rhs=h_all[:, 2 * p2:2 * p2 + 2, isl],
                                 start=(p2 == 0), stop=(p2 == 1), perf_mode=DR)
            if j % 2 == 0:
                expair = exp_pool.tile([P, 2, NB], F8, name=f"ex_{ib}_{j}",
                                       tag="exp")
            nc.scalar.activation(out=expair[:, j % 2, :], in_=sc,
                                 func=mybir.ActivationFunctionType.Exp,
                                 bias=expbias_sb, scale=SCALE / WSCALE)
            if pending is not None and j == 1:
                # previous block's epilogue slots in here, before this
                # block's first att matmul reuses the accumulator banks
                emit_tail(*pending)
                pending = None
            if j % 2 == 1:
                jp = j - 1
                # softmax denominator: all-16s stationary reduces over the
                # 256 key positions and broadcasts S to all 128 partitions.
                # On the final pair S goes first so the reciprocal overlaps
                # the remaining att matmuls.
                def emit_s():
                    nc.tensor.matmul(s_ps, lhsT=ones16, rhs=expair,
                                     start=(jp == 0), stop=(jp == NJ - 2),
                                     perf_mode=DR)
                if jp == NJ - 2:
                    emit_s()
                for c in range(CT):
                    nc.tensor.matmul(att_ps[c],
                                     lhsT=ut[:, jp:jp + 2, c * P:(c + 1) * P],
                                     rhs=expair,
                                     start=(jp == 0), stop=(jp == NJ - 2),
                                     perf_mode=DR)
                if jp != NJ - 2:
                    emit_s()
        pending = (ib, att_ps, s_ps)
    emit_tail(*pending, last=True)


def _emit_general(ctx: ExitStack, tc: tile.TileContext):
    """Fallback: separate q/k convs with biases, fp16 operands (baseline)."""
    nc = tc.nc
    x_d = nc.declare_dram_parameter("x", [C, N], F32, isOutput=False)
    wqT_d = nc.declare_dram_parameter("wqT", [C, C], F16, isOutput=False)
    wkT_d = nc.declare_dram_parameter("wkT", [C, C], F16, isOutput=False)
    wvT_d = nc.declare_dram_parameter("wvT", [C, C], F16, isOutput=False)
    wpT_d = nc.declare_dram_parameter("wpT", [C, C], F16, isOutput=False)
    bq_d = nc.declare_dram_parameter("bq", [C], F32, isOutput=False)
    bk_d = nc.declare_dram_parameter("bk", [C], F32, isOutput=False)
    bp2_d = nc.declare_dram_parameter("bp2", [C], F32, isOutput=False)
    gamma_d = nc.declare_dram_parameter("gamma", [C], F32, isOutput=False)
    beta_d = nc.declare_dram_parameter("beta", [C], F32, isOutput=False)
    mask_d = nc.declare_dram_parameter("gmask", [P, GPT], F32, isOutput=False)
    expand_d = nc.declare_dram_parameter("gexpand", [GPT, P], F32, isOutput=False)
    out_d = nc.declare_dram_parameter("out", [C, NQ], F32, isOutput=True)

    consts = ctx.enter_context(tc.tile_pool(name="consts", bufs=1))
    big = ctx.enter_context(tc.tile_pool(name="big", bufs=1))
    stage = ctx.enter_context(tc.tile_pool(name="stage", bufs=2))
    gn_small = ctx.enter_context(tc.tile_pool(name="gn_small", bufs=2))
    exp_pool = ctx.enter_context(tc.tile_pool(name="exp_pool", bufs=4))
    att_sb_pool = ctx.enter_context(tc.tile_pool(name="att_sb_pool", bufs=2))
    out_pool = ctx.enter_context(tc.tile_pool(name="out_pool", bufs=4))
    ps_mm = ctx.enter_context(tc.tile_pool(name="ps_mm", bufs=4, space="PSUM"))
    ps_att = ctx.enter_context(tc.tile_pool(name="ps_att", bufs=1, space="PSUM"))

    ident_f = mybir.ActivationFunctionType.Identity

    xs_tiles = []
    for t in range(CT):
        xs = stage.tile([P, N], F32, name=f"xs_{t}", tag="xs")
        for ch in range(4):
            nc.sync.dma_start(out=xs[:, ch * (N // 4):(ch + 1) * (N // 4)],
                              in_=x_d[t * P:(t + 1) * P,
                                      ch * (N // 4):(ch + 1) * (N // 4)])
        xs_tiles.append(xs)

    mask_sb = consts.tile([P, GPT], F32, name="mask_sb", tag="mask_sb")
    nc.gpsimd.dma_start(out=mask_sb, in_=mask_d[:, :])
    expand_sb = consts.tile([GPT, P], F32, name="expand_sb", tag="expand_sb")
    nc.gpsimd.dma_start(out=expand_sb, in_=expand_d[:, :])

    def load_vec(ap, nm):
        r = ap[:].rearrange("(t p) -> t p", p=P)
        tiles = []
        for t in range(CT):
            tl = consts.tile([P, 1], F32, name=f"{nm}_{t}", tag=f"{nm}_{t}")
            nc.gpsimd.dma_start(out=tl, in_=r[t][:, None])
            tiles.append(tl)
        return tiles

    gamma_sb = load_vec(gamma_d, "gamma")
    beta_sb = load_vec(beta_d, "beta")
    bq_sb = load_vec(bq_d, "bq")
    bk_sb = load_vec(bk_d, "bk")
    bp2_sb = load_vec(bp2_d, "bp2")

    w_sb = {}
    w_order = (("k", wkT_d), ("v", wvT_d), ("q", wqT_d), ("p", wpT_d))
    for wname, w_ap in w_order:
        for t in range(CT):
            tl = consts.tile([P, C], F16, name=f"w{wname}_{t}", tag=f"w{wname}_{t}")
            nc.sync.dma_start(out=tl, in_=w_ap[t * P:(t + 1) * P, :])
            w_sb[wname, t] = tl
    ones32 = consts.tile([P, P], F32, name="ones32", tag="ones32")
    nc.vector.memset(ones32, 1.0)
    expbias_sb = consts.tile([P, 1], F32, name="expbias_sb", tag="expbias_sb")
    nc.vector.memset(expbias_sb, -4.0)

    h_sb = [big.tile([P, N], F16, name=f"h_{t}", tag=f"h_{t}") for t in range(CT)]
    k_sb = [big.tile([P, N], F16, name=f"k_{t}", tag=f"k_{t}") for t in range(CT)]
    q_sb = [big.tile([P, NQ], F16, name=f"q_{t}", tag=f"q_{t}") for t in range(CT)]
    vt_sb = big.tile([P, NJ, C], F16, name="vt_sb", tag="vt_sb")

    for t in range(CT):
        xs = xs_tiles[t]
        st = gn_small.tile([P, N // NB, 6], F32, name=f"st_{t}", tag="st")
        xs_c = xs.rearrange("p (c f) -> p c f", f=NB)
        for cchunk in range(N // NB):
            nc.vector.bn_stats(out=st[:, cchunk, :], in_=xs_c[:, cchunk, :])
        ms2 = gn_small.tile([P, 2], F32, name=f"ms2_{t}", tag="ms2")
        nc.vector.bn_aggr(out=ms2, in_=st)
        msq = gn_small.tile([P, 1], F32, name=f"msq_{t}", tag="msq")
        nc.gpsimd.tensor_tensor(msq, ms2[:, 0:1], ms2[:, 0:1],
                                mybir.AluOpType.mult)
        nc.gpsimd.tensor_add(ms2[:, 1:2], ms2[:, 1:2], msq)
        gps = ps_mm.tile([GPT, 2], F32, name=f"gps_{t}", tag="mm")
        nc.tensor.matmul(gps, lhsT=mask_sb, rhs=ms2, start=True, stop=True)
        gmv = gn_small.tile([GPT, 2], F32, name=f"gmv_{t}", tag="gmv")
        nc.vector.tensor_copy(out=gmv, in_=gps)
        vpe = gn_small.tile([GPT, 1], F32, name=f"vpe_{t}", tag="vpe")
        nc.gpsimd.tensor_tensor(vpe, gmv[:, 0:1], gmv[:, 0:1], mybir.AluOpType.mult)
        nc.gpsimd.tensor_scalar(vpe, gmv[:, 1:2], vpe, EPS,
                                mybir.AluOpType.subtract, mybir.AluOpType.add)
        sd = gn_small.tile([GPT, 1], F32, name=f"sd_{t}", tag="sd")
        nc.scalar.sqrt(out=sd, in_=vpe)
        y0 = gn_small.tile([GPT, 1], F32, name=f"y0_{t}", tag="y0")
        nc.vector.reciprocal(out=y0, in_=sd)
        t1 = gn_small.tile([GPT, 1], F32, name=f"t1_{t}", tag="t1")
        nc.gpsimd.tensor_tensor(t1, y0, y0, mybir.AluOpType.mult)
        nc.gpsimd.tensor_tensor(t1, t1, vpe, mybir.AluOpType.mult)
        nc.gpsimd.tensor_scalar(t1, t1, -0.5, 1.5,
                                mybir.AluOpType.mult, mybir.AluOpType.add)
        grs = gn_small.tile([GPT, 2], F32, name=f"grs_{t}", tag="grs")
        nc.gpsimd.tensor_copy(out=grs[:, 0:1], in_=gmv[:, 0:1])
        nc.gpsimd.tensor_tensor(grs[:, 1:2], y0, t1, mybir.AluOpType.mult)
        cps = ps_mm.tile([P, 2], F32, name=f"cps_{t}", tag="mm")
        nc.tensor.matmul(cps, lhsT=expand_sb, rhs=grs, start=True, stop=True)
        cms = gn_small.tile([P, 2], F32, name=f"cms_{t}", tag="cms")
        nc.vector.tensor_copy(out=cms, in_=cps)
        a_t = gn_small.tile([P, 1], F32, name=f"a_{t}", tag="a")
        nc.gpsimd.tensor_tensor(a_t, gamma_sb[t], cms[:, 1:2], mybir.AluOpType.mult)
        b_t = gn_small.tile([P, 1], F32, name=f"b_{t}", tag="b")
        nc.gpsimd.tensor_tensor(b_t, cms[:, 0:1], a_t, mybir.AluOpType.mult)
        nc.gpsimd.tensor_tensor(b_t, beta_sb[t], b_t, mybir.AluOpType.subtract)
        nc.scalar.activation(out=h_sb[t][:, :N // 2], in_=xs[:, :N // 2],
                             func=ident_f, bias=b_t, scale=a_t)
        nc.vector.tensor_scalar(h_sb[t][:, N // 2:], xs[:, N // 2:], a_t, b_t,
                                mybir.AluOpType.mult, mybir.AluOpType.add)

    conv_n = 0

    def conv_psum(nm, free):
        nonlocal conv_n
        conv_n += 1
        if conv_n % 8 < 4:
            return ps_mm.tile([P, free], F32, name=nm, tag="mm")
        return ps_att.tile([P, free], F32, name=nm, tag=f"att{conv_n % 8 - 4}")

    for co in range(CT):
        for nb in range(N // NB):
            ps = conv_psum(f"kps_{co}_{nb}", NB)
            for ci in range(CT):
                nc.tensor.matmul(ps, lhsT=w_sb["k", ci][:, co * P:(co + 1) * P],
                                 rhs=h_sb[ci][:, nb * NB:(nb + 1) * NB],
                                 start=(ci == 0), stop=(ci == CT - 1))
            nc.scalar.activation(out=k_sb[co][:, nb * NB:(nb + 1) * NB],
                                 in_=ps, func=ident_f, bias=bk_sb[co], scale=1.0)
    for co in range(CT):
        for nb in range(NQ // NB):
            ps = conv_psum(f"qps_{co}_{nb}", NB)
            for ci in range(CT):
                nc.tensor.matmul(ps,
                                 lhsT=w_sb["q", ci][:, co * P:(co + 1) * P],
                                 rhs=h_sb[ci][:, nb * NB:(nb + 1) * NB],
                                 start=(ci == 0), stop=(ci == CT - 1))
            nc.scalar.activation(out=q_sb[co][:, nb * NB:(nb + 1) * NB],
                                 in_=ps, func=ident_f, bias=bq_sb[co],
                                 scale=1.0)
    for j in range(NJ):
        ps = conv_psum(f"vps_{j}", C)
        for ci in range(CT):
            nc.tensor.matmul(ps, lhsT=h_sb[ci][:, j * P:(j + 1) * P],
                             rhs=w_sb["v", ci],
                             start=(ci == 0), stop=(ci == CT - 1))
        nc.scalar.copy(out=vt_sb[:, j, :], in_=ps)

    def emit_tail(ib, att_ps, sacc):
        isl = slice(ib * NB, (ib + 1) * NB)
        sps = ps_mm.tile([P, NB], F32, name=f"sps_{ib}", tag="mm")
        nc.tensor.matmul(sps, lhsT=ones32, rhs=sacc, start=True, stop=True)
        rb = out_pool.tile([P, NB], F32, name=f"rb_{ib}", tag="rb", bufs=2)
        rscr = out_pool.tile([P, NB], F32, name=f"rscr_{ib}", tag="rscr", bufs=2)
        nc.vector.reciprocal_approx_accurate(out=rb, in_=sps, scratch=rscr)
        att_sb = []
        for c in range(CT):
            asb = att_sb_pool.tile([P, NB], F16, name=f"attsb_{ib}_{c}",
                                   tag=f"asb{c}")
            nc.scalar.copy(out=asb, in_=att_ps[c])
            att_sb.append(asb)
        for co in range(CT):
            xres = out_pool.tile([P, NB], F32, name=f"xres_{ib}_{co}", tag="xres")
            nc.gpsimd.dma_start(out=xres, in_=x_d[co * P:(co + 1) * P, isl])
            pp = ps_mm.tile([P, NB], F32, name=f"pp_{ib}_{co}", tag="mm")
            for ci in range(CT):
                nc.tensor.matmul(pp, lhsT=w_sb["p", ci][:, co * P:(co + 1) * P],
                                 rhs=att_sb[ci],
                                 start=(ci == 0), stop=(ci == CT - 1))
            fin = out_pool.tile([P, NB], F32, name=f"fin_{ib}_{co}", tag="fin")
            for hh in range(2):
                hs = slice(hh * (NB // 2), (hh + 1) * (NB // 2))
                nc.vector.tensor_tensor(fin[:, hs], pp[:, hs], rb[:, hs],
                                        mybir.AluOpType.mult)
                nc.vector.tensor_scalar_add(fin[:, hs], fin[:, hs], bp2_sb[co])
                nc.vector.tensor_add(fin[:, hs], fin[:, hs], xres[:, hs])
                nc.sync.dma_start(
                    out=out_d[co * P:(co + 1) * P,
                              ib * NB + hh * (NB // 2):
                              ib * NB + (hh + 1) * (NB // 2)],
                    in_=fin[:, hs])

    pending = None
    for ib in range(NQ // NB):
        isl = slice(ib * NB, (ib + 1) * NB)
        att_ps = [ps_att.tile([P, NB], F32, name=f"attps_{ib}_{c}", tag=f"att{c}")
                  for c in range(CT)]
        sacc = out_pool.tile([P, NB], F32, name=f"sacc_{ib}", tag="sacc", bufs=2)
        ex_tiles = {}
        for j in range(NJ + 1):
            if j < NJ:
                sc = ps_mm.tile([P, NB], F32, name=f"sc_{ib}_{j}", tag="mm")
                for ci in range(CT):
                    nc.tensor.matmul(sc, lhsT=k_sb[ci][:, j * P:(j + 1) * P],
                                     rhs=q_sb[ci][:, isl],
                                     start=(ci == 0), stop=(ci == CT - 1))
                ex = exp_pool.tile([P, NB], F16, name=f"ex_{ib}_{j}", tag="exp")
                nc.scalar.activation(out=ex, in_=sc,
                                     func=mybir.ActivationFunctionType.Exp,
                                     bias=expbias_sb, scale=SCALE)
                ex_tiles[j] = ex
            if pending is not None and j == 1:
                emit_tail(*pending)
                pending = None
            if j >= 1:
                jp = j - 1
                ex = ex_tiles.pop(jp)
                for c in range(CT):
                    nc.tensor.matmul(att_ps[c],
                                     lhsT=vt_sb[:, jp, c * P:(c + 1) * P],
                                     rhs=ex, start=(jp == 0), stop=(jp == NJ - 1))
                if jp == 0:
                    nc.vector.tensor_copy(out=sacc, in_=ex)
                else:
                    nc.vector.tensor_add(sacc, sacc, ex)
        pending = (ib, att_ps, sacc)
    emit_tail(*pending)


_CACHED = {}


def _build(merged=True, bias0=True):
    key = ("fp8", bias0) if merged else ("gen",)
    if key not in _CACHED:
        nc = bacc.Bacc()
        with tile.TileContext(nc) as tc, ExitStack() as ctx:
            if merged:
                _emit_fp8(ctx, tc, bias0)
            else:
                _emit_general(ctx, tc)
        nc.finalize()
        _CACHED[key] = nc
    return _CACHED[key]


def _to_f8(a):
    return np.asarray(np.clip(a, -240.0, 240.0),
                      dtype=ml_dtypes.float8_e4m3)


def _pack_dr(wT):
    # [C_in, C_out] -> [128, 2(pair), 2(elem), C_out] for DoubleRow lhsT/rhs
    return np.ascontiguousarray(
        np.transpose(wT.reshape(2, 2, P, C), (2, 0, 1, 3)))


def _host_inputs(x, norm_gamma, norm_beta, Wq, bq, Wk, bk, Wv, bv, Wp, bp,
                 merged=None):
    if merged is None:
        merged = (not np.any(np.asarray(bq))) and (not np.any(np.asarray(bk)))
    bp2 = (np.asarray(Wp, np.float64) @ np.asarray(bv, np.float64)
           + np.asarray(bp, np.float64)).astype(np.float32)
    common = {
        "gamma": np.asarray(norm_gamma, np.float32),
        "beta": np.asarray(norm_beta, np.float32),
        "gmask": ((np.arange(P)[:, None] // GS == np.arange(GPT)[None, :])
                  .astype(np.float32) / GS),
        "gexpand": (np.arange(GPT)[:, None] == np.arange(P)[None, :] // GS)
                   .astype(np.float32),
    }
    if merged:
        bias0 = not np.any(bp2)
        wmT = (np.asarray(Wk, np.float64).T @ np.asarray(Wq, np.float64))
        wuT = (np.asarray(Wp, np.float64) @ np.asarray(Wv, np.float64)).T
        common["wmk"] = _pack_dr(_to_f8(wmT * WSCALE))
        common["wuv"] = _pack_dr(_to_f8(wuT * WSCALE))
        if not bias0:
            common["bp2"] = bp2
    else:
        common["wqT"] = np.ascontiguousarray(
            np.asarray(Wq, np.float32).T).astype(np.float16)
        common["wkT"] = np.ascontiguousarray(
            np.asarray(Wk, np.float32).T).astype(np.float16)
        common["wvT"] = np.ascontiguousarray(
            np.asarray(Wv, np.float32).T).astype(np.float16)
        common["wpT"] = np.ascontiguousarray(
            np.asarray(Wp, np.float32).T).astype(np.float16)
        common["bq"] = np.asarray(bq, np.float32)
        common["bk"] = np.asarray(bk, np.float32)
        common["bp2"] = bp2
    xf = np.asarray(x, np.float32).reshape(4, C, N)
    if merged:
        xf = xf.astype(np.float16)
    in_maps = []
    for core in range(N_CORES):
        bi, qh = core // 2, core % 2
        xc = np.ascontiguousarray(np.roll(xf[bi], -qh * NQ, axis=1))
        in_maps.append({"x": xc, **common})
    return in_maps


def kernel(x, norm_gamma, norm_beta, Wq, bq, Wk, bk, Wv, bv, Wp, bp):
    x = np.asarray(x, np.float32)
    b, c, hh, ww = x.shape
    assert (b, c, hh * ww) == (4, C, N)
    merged = (not np.any(np.asarray(bq))) and (not np.any(np.asarray(bk)))
    bp2 = np.asarray(Wp, np.float64) @ np.asarray(bv, np.float64) \
        + np.asarray(bp, np.float64)
    bias0 = not np.any(bp2)
    nc = _build(merged, bias0)
    in_maps = _host_inputs(x, norm_gamma, norm_beta,
                           Wq, bq, Wk, bk, Wv, bv, Wp, bp, merged=merged)
    res = run_bass_kernel_spmd(nc, in_maps, core_ids=list(range(N_CORES)))
    y = np.empty((4, C, N), np.float32)
    for core in range(N_CORES):
        bi, qh = core // 2, core % 2
        y[bi][:, qh * NQ:(qh + 1) * NQ] = \
            res.results[core]["out"].astype(np.float32)
    return y.reshape(b, c, hh, ww)
